# revision 11
# baseline (speedup 1.0000x reference)
"""Multi-head attention TRN2 kernel, 8-core (batch x head-block) sharded.

Problem (hardcoded): x[2,2048,1024] f32, Wq/Wk/Wv[1024,1024], Wo[1024,1024],
16 heads, dh=64. Reference computes softmax(Q K^T)/sqrt(1024) @ V @ Wo with the
division AFTER softmax (folded here into Wo as a host-side 1/32 scale).

Sharding: core c handles batch b=c//4 and head block hb=c%4 (4 heads = 256 dims:
Wq/Wk/Wv column slice, Wo row slice). Each core emits a partial Y[2048,1024];
host sums the 4 partials per batch.

v2 layout/engine plan:
- Host pre-swizzles x and weights into SBUF-shaped DRAM tensors so every load
  is one big contiguous DMA (8 total + 16 output stores).
- Q/K kept dense in bf16: head pair (2g, 2g+1) lives on partitions 0:64/64:128
  of one SBUF tile, so the two K=64 QK matmuls pack into disjoint PE row
  groups (tile_position (0,0)/(64,0) inferred from base partitions) and run
  concurrently.
- V is interleaved with a ones column block ([tok, 64 V | 64 ones]) so one PV
  matmul accumulates both the numerator O and the softmax denominator.
- Softmax normalization: reciprocal_approx_fast (1 DVE op, ~18-bit) + one
  tensor_tensor multiply; no Newton iterations, no 6.6us DVE reciprocal.
- ACT does exp only (the pacing engine, ~147us); psum->sbuf copies go to DVE
  except the first two x-chunks where ACT is otherwise idle.
"""

import numpy as np

import concourse.tile as tile
from concourse import bacc, mybir
from concourse.bass_utils import run_bass_kernel_spmd

N_CORES = 8
B = 2
S = 2048          # tokens per batch (= per core)
D = 1024          # model dim
DH = 64           # head dim
HPC = 4           # heads per core
DL = HPC * DH     # 256 local qkv dims per core
NG = DL // 128    # 2 head-pair groups
NK = D // 128     # 8 k-strips for QKV contraction
NT = S // 128     # 16 key strips
NC = S // 512     # 4 token chunks

F32 = mybir.dt.float32
DT = mybir.dt.float32r   # PE fast fp32 mode
BF = mybir.dt.bfloat16
EXP = mybir.ActivationFunctionType.Exp
MULT = mybir.AluOpType.mult


def build_nc(dumps=False):
    nc = bacc.Bacc("TRN2", target_bir_lowering=False, debug=False)
    xC = nc.declare_dram_parameter("xC", [128, NC * 4096], DT, isOutput=False)
    Wq = nc.declare_dram_parameter("Wq", [128, NK * 256], DT, isOutput=False)
    Wk = nc.declare_dram_parameter("Wk", [128, NK * 256], DT, isOutput=False)
    Wv = nc.declare_dram_parameter("Wv", [128, NK * 256], DT, isOutput=False)
    Wo = nc.declare_dram_parameter("Wo", [128, NG * D], BF, isOutput=False)
    Yp = nc.declare_dram_parameter("Yp", [S, D], F32, isOutput=True)
    if dumps:
        d_qt = nc.declare_dram_parameter("d_qt", [128, NG * S], BF,
                                         isOutput=True)
        d_kt = nc.declare_dram_parameter("d_kt", [128, NG * S], BF,
                                         isOutput=True)
        d_va = nc.declare_dram_parameter("d_va", [128, HPC * NT * 128], BF,
                                         isOutput=True)
        d_ot = nc.declare_dram_parameter("d_ot", [128, NG * S], BF,
                                         isOutput=True)
        d_ps = nc.declare_dram_parameter("d_ps", [128, 1024], F32,
                                         isOutput=True)
        d_ps2 = nc.declare_dram_parameter("d_ps2", [128, 1024], F32,
                                          isOutput=True)
        d_po = nc.declare_dram_parameter("d_po", [128, 1024], F32,
                                         isOutput=True)

    with tile.TileContext(nc) as tc:
        with tc.tile_pool(name="singles", bufs=1) as singles:
            wq_sb = singles.tile([128, NK * 256], DT)
            wk_sb = singles.tile([128, NK * 256], DT)
            wv_sb = singles.tile([128, NK * 256], DT)
            wo_sb = singles.tile([128, NG * D], BF)
            # qt/kt: dim-major, bf16. Group g: rows 0:64 = head 2g, rows
            # 64:128 = head 2g+1 (this is what lets QK row-pack the PE).
            qt_sb = singles.tile([128, NG * S], BF)
            kt_sb = singles.tile([128, NG * S], BF)
            ot_sb = singles.tile([128, NG * S], BF)
            # vaug: token-major V, bf16. Head block h, strip j at cols
            # h*2048 + j*128; within a block cols 0:64 = V dims, 64:128 = ones
            # (PV matmul then emits numerator rows 0:64, denominator 64:128).
            vaug_sb = singles.tile([128, HPC * NT * 128], BF)

            # ---- weight DMAs + ones fill --------------------------------
            nc.sync.dma_start(out=wq_sb[:], in_=Wq[:, :])
            nc.sync.dma_start(out=wk_sb[:], in_=Wk[:, :])
            nc.sync.dma_start(out=wv_sb[:], in_=Wv[:, :])
            nc.sync.dma_start(out=wo_sb[:], in_=Wo[:, :])
            for h in range(HPC):
                ones_view = vaug_sb[:, h * 2048:(h + 1) * 2048].rearrange(
                    "p (j b c) -> p j b c", j=NT, b=2, c=64)[:, :, 1, :]
                nc.gpsimd.memset(ones_view, 1.0)

            # ---- phase 1: QKV projections -------------------------------
            # qk_unit / v_unit emit one PSUM tile's worth of projection work
            # (8 accumulating matmuls + one evacuation copy). Chunks 0/1 run
            # up front; chunks 2/3 are emitted as filler units inside the
            # first attention head's strip loop to keep the PE duty cycle
            # high (HAM clock gate) and hide phase-1 time under ACT exp.
            def qk_unit(xc, c, w_sb, dst, g, use_act, pp1):
                ps = pp1.tile([128, 512], F32, name="ps_qk")
                for k in range(NK):
                    nc.tensor.matmul(
                        ps[:],
                        w_sb[:, (k * NG + g) * 128:(k * NG + g + 1) * 128],
                        xc[:, k * 512:(k + 1) * 512],
                        start=(k == 0),
                        stop=(k == NK - 1),
                    )
                dst_ap = dst[:, g * S + c * 512:g * S + (c + 1) * 512]
                if use_act:
                    nc.scalar.copy(out=dst_ap, in_=ps[:])
                else:
                    nc.vector.tensor_copy(out=dst_ap, in_=ps[:])

            def v_unit(xc, c, t, ppv):
                j = c * 4 + t
                pv = ppv.tile([128, DL], F32, name="pv")
                for k in range(NK):
                    nc.tensor.matmul(
                        pv[:],
                        xc[:, k * 512 + t * 128:k * 512 + (t + 1) * 128],
                        wv_sb[:, k * DL:(k + 1) * DL],
                        start=(k == 0),
                        stop=(k == NK - 1),
                    )
                dst = vaug_sb[:].rearrange(
                    "p (h j b c) -> p h j b c",
                    h=HPC, j=NT, b=2, c=64)[:, :, j, 0, :]
                nc.vector.tensor_copy(
                    out=dst, in_=pv[:].rearrange("p (h c) -> p h c", h=HPC))

            def load_chunk(c, xcp):
                xc = xcp.tile([128, 4096], DT, name="xc")
                nc.sync.dma_start(out=xc[:], in_=xC[:, c * 4096:(c + 1) * 4096])
                return xc

            # ---- phase 2/3 emitters -------------------------------------
            def emit_pv(pO, expst, h, j):
                vb = h * 2048 + j * 128
                for sc in range(2):
                    nc.tensor.matmul(
                        pO[:, sc * 512:(sc + 1) * 512],
                        vaug_sb[:, vb:vb + 128],
                        expst[:, sc * 512:(sc + 1) * 512],
                        start=(j == 0),
                        stop=(j == NT - 1),
                        skip_group_check=True,
                    )

            def emit_norm(normp, pO, h, sh):
                g, r = h // 2, (h % 2) * DH
                rb = normp.tile([DH, 1024], F32, name="rb")
                # stock DVE reciprocal is bit-exact iterative divide
                # (~6 cyc/elem); reciprocal_approx_fast mis-executes in
                # this environment (custom ucode row unavailable).
                nc.vector.reciprocal(rb[:], pO[DH:128, :])
                nc.vector.tensor_tensor(
                    out=ot_sb[r:r + DH,
                              g * S + sh * 1024:g * S + (sh + 1) * 1024],
                    in0=pO[0:DH, :],
                    in1=rb[:],
                    op=MULT,
                )

            def ph3_unit(t, pYp, ysbp):
                pY = pYp.tile([128, 1024], F32, name="pY")
                for e in range(2):
                    for g in range(NG):
                        nc.tensor.matmul(
                            pY[:, e * 512:(e + 1) * 512],
                            ot_sb[:, g * S + t * 128:g * S + (t + 1) * 128],
                            wo_sb[:, g * D + e * 512:g * D + (e + 1) * 512],
                            start=(g == 0),
                            stop=(g == NG - 1),
                        )
                ysb = ysbp.tile([128, 1024], F32, name="ysb")
                nc.vector.tensor_copy(out=ysb[:], in_=pY[:])
                nc.sync.dma_start(out=Yp[t * 128:(t + 1) * 128, :], in_=ysb[:])

            def ph2_head(h, sh, pSp, pOp, expp, normp, filler=()):
                filler = list(filler)
                pr, hb = h // 2, (h % 2) * 64
                pO = pOp.tile([128, 1024], F32, name="pO")
                prev = None
                for j in range(NT):
                    pS = pSp.tile([128, 1024], F32, name="pS")
                    for sc in range(2):
                        q0 = pr * S + sh * 1024 + sc * 512
                        nc.tensor.matmul(
                            pS[:, sc * 512:(sc + 1) * 512],
                            kt_sb[hb:hb + 64,
                                  pr * S + j * 128:pr * S + (j + 1) * 128],
                            qt_sb[hb:hb + 64, q0:q0 + 512],
                        )
                    expst = expp.tile([128, 1024], BF, name="expst")
                    nc.scalar.activation(expst[:], pS[:], EXP)
                    if prev is not None:
                        emit_pv(pO, prev, h, j - 1)
                    prev = expst
                    if filler:
                        filler.pop(0)()
                emit_pv(pO, prev, h, NT - 1)
                for f in filler:
                    f()
                emit_norm(normp, pO, h, sh)

            with tc.tile_pool(name="pS", bufs=2, space="PSUM") as pSp, \
                 tc.tile_pool(name="pO", bufs=1, space="PSUM") as pOp, \
                 tc.tile_pool(name="expp", bufs=4) as expp, \
                 tc.tile_pool(name="normp", bufs=2) as normp:
                with tc.tile_pool(name="xcp", bufs=2) as xcp, \
                     tc.tile_pool(name="pp1", bufs=1, space="PSUM") as pp1, \
                     tc.tile_pool(name="ppv", bufs=2, space="PSUM") as ppv:
                    for c in (0, 1):
                        xc = load_chunk(c, xcp)
                        for w_sb, dst in ((wq_sb, qt_sb), (wk_sb, kt_sb)):
                            for g in range(NG):
                                qk_unit(xc, c, w_sb, dst, g, True, pp1)
                        for t in range(4):
                            v_unit(xc, c, t, ppv)
                    # chunks 2/3 become 16 filler units inside head 0, sh 0.
                    # K/V first (needed by strips j>=8), Q (sh=1) last.
                    xc2, xc3 = load_chunk(2, xcp), load_chunk(3, xcp)
                    filler = []
                    for xc, c in ((xc2, 2), (xc3, 3)):
                        for g in range(NG):
                            filler.append(lambda xc=xc, c=c, g=g: qk_unit(
                                xc, c, wk_sb, kt_sb, g, False, pp1))
                        for t in range(4):
                            filler.append(lambda xc=xc, c=c, t=t: v_unit(
                                xc, c, t, ppv))
                    for xc, c in ((xc2, 2), (xc3, 3)):
                        for g in range(NG):
                            filler.append(lambda xc=xc, c=c, g=g: qk_unit(
                                xc, c, wq_sb, qt_sb, g, False, pp1))
                    ph2_head(0, 0, pSp, pOp, expp, normp, filler)
                    for h in (1, 2, 3):
                        ph2_head(h, 0, pSp, pOp, expp, normp)
                with tc.tile_pool(name="pYp", bufs=1, space="PSUM") as pYp, \
                     tc.tile_pool(name="ysbp", bufs=2) as ysbp:
                    ph3f = [lambda t=t: ph3_unit(t, pYp, ysbp)
                            for t in range(8)]
                    ph2_head(0, 1, pSp, pOp, expp, normp, ph3f)
                    for h in (1, 2, 3):
                        ph2_head(h, 1, pSp, pOp, expp, normp)
                    for t in range(8, 16):
                        ph3_unit(t, pYp, ysbp)
                if dumps:
                    nc.sync.dma_start(out=d_qt[:, :], in_=qt_sb[:])
                    nc.sync.dma_start(out=d_kt[:, :], in_=kt_sb[:])
                    nc.sync.dma_start(out=d_va[:, :], in_=vaug_sb[:])
                    nc.sync.dma_start(out=d_ot[:, :], in_=ot_sb[:])
    nc.finalize()
    return nc


def make_in_maps(x, Wq, Wk, Wv, Wo):
    f = np.float32
    import ml_dtypes
    bf = ml_dtypes.bfloat16
    x = np.asarray(x, f)
    Wq, Wk, Wv, Wo = (np.asarray(a, f) for a in (Wq, Wk, Wv, Wo))
    in_maps = []
    # xC: chunk-major k-strips of x^T: xC[p, c*4096 + k*512 + j]
    #   = x[b, c*512 + j, k*128 + p]
    xCs = []
    for b in range(B):
        xT = np.ascontiguousarray(x[b].T)               # [1024, 2048]
        xC = xT.reshape(NK, 128, NC, 512).transpose(1, 2, 0, 3)
        xCs.append(np.ascontiguousarray(xC.reshape(128, NC * 4096)))

    def wsw(W):  # [1024, 256] -> [128, 2048], block (k*2+g)*128
        return np.ascontiguousarray(
            W.reshape(NK, 128, NG, 128).transpose(1, 0, 2, 3)
            .reshape(128, NK * 256))

    for c in range(N_CORES):
        b, hb = divmod(c, N_CORES // B)
        cols = slice(hb * DL, (hb + 1) * DL)
        wo = (Wo[cols, :] * f(1.0 / 32.0)).reshape(NG, 128, D)
        wo = np.ascontiguousarray(
            wo.transpose(1, 0, 2).reshape(128, NG * D)).astype(bf)
        in_maps.append({
            "xC": xCs[b],
            "Wq": wsw(Wq[:, cols]),
            "Wk": wsw(Wk[:, cols]),
            "Wv": wsw(Wv[:, cols]),
            "Wo": wo,
        })
    return in_maps


def run(inputs, trace=False):
    nc = build_nc()
    in_maps = make_in_maps(**inputs)
    res = run_bass_kernel_spmd(nc, in_maps, list(range(N_CORES)), trace=trace)
    yps = [res.results[c]["Yp"] for c in range(N_CORES)]
    out = np.empty((B, S, D), np.float32)
    cpb = N_CORES // B
    for b in range(B):
        out[b] = sum(yps[b * cpb:(b + 1) * cpb])
    return out, res


def kernel(**inputs):
    out, _ = run(inputs, trace=False)
    return out


# revision 14
# speedup vs baseline: 1.3536x; 1.3536x over previous
"""Multi-head attention TRN2 kernel, 8-core (batch x head-block) sharded.

Problem (hardcoded): x[2,2048,1024] f32, Wq/Wk/Wv[1024,1024], Wo[1024,1024],
16 heads, dh=64. Reference computes softmax(Q K^T)/sqrt(1024) @ V @ Wo with the
division AFTER softmax (folded here into Wo as a host-side 1/32 scale).

Sharding: core c handles batch b=c//4 and head block hb=c%4 (4 heads = 256 dims:
Wq/Wk/Wv column slice, Wo row slice). Each core emits a partial Y[2048,1024];
host sums the 4 partials per batch.

v2 layout/engine plan:
- Host pre-swizzles x and weights into SBUF-shaped DRAM tensors so every load
  is one big contiguous DMA (8 total + 16 output stores).
- Q/K kept dense in bf16: head pair (2g, 2g+1) lives on partitions 0:64/64:128
  of one SBUF tile, so the two K=64 QK matmuls pack into disjoint PE row
  groups (tile_position (0,0)/(64,0) inferred from base partitions) and run
  concurrently.
- V is interleaved with a ones column block ([tok, 64 V | 64 ones]) so one PV
  matmul accumulates both the numerator O and the softmax denominator.
- Softmax normalization: reciprocal_approx_fast (1 DVE op, ~18-bit) + one
  tensor_tensor multiply; no Newton iterations, no 6.6us DVE reciprocal.
- ACT does exp only (the pacing engine, ~147us); psum->sbuf copies go to DVE
  except the first two x-chunks where ACT is otherwise idle.
"""

import numpy as np

import concourse.tile as tile
from concourse import bacc, mybir
from concourse.bass_utils import run_bass_kernel_spmd

N_CORES = 8
B = 2
S = 2048          # tokens per batch (= per core)
D = 1024          # model dim
DH = 64           # head dim
HPC = 4           # heads per core
DL = HPC * DH     # 256 local qkv dims per core
NG = DL // 128    # 2 head-pair groups
NK = D // 128     # 8 k-strips for QKV contraction
NT = S // 128     # 16 key strips
NC = S // 512     # 4 token chunks

F32 = mybir.dt.float32
DT = mybir.dt.float32r   # PE fast fp32 mode
BF = mybir.dt.bfloat16
EXP = mybir.ActivationFunctionType.Exp
MULT = mybir.AluOpType.mult


def build_nc(dumps=False):
    nc = bacc.Bacc("TRN2", target_bir_lowering=False, debug=False)
    xC = nc.declare_dram_parameter("xC", [128, NC * 4096], DT, isOutput=False)
    Wq = nc.declare_dram_parameter("Wq", [128, NK * 256], DT, isOutput=False)
    Wk = nc.declare_dram_parameter("Wk", [128, NK * 256], DT, isOutput=False)
    Wv = nc.declare_dram_parameter("Wv", [128, NK * 256], DT, isOutput=False)
    Wo = nc.declare_dram_parameter("Wo", [128, NG * D], BF, isOutput=False)
    Yp = nc.declare_dram_parameter("Yp", [S, D], F32, isOutput=True)
    if dumps:
        d_qt = nc.declare_dram_parameter("d_qt", [128, NG * S], BF,
                                         isOutput=True)
        d_kt = nc.declare_dram_parameter("d_kt", [128, NG * S], BF,
                                         isOutput=True)
        d_va = nc.declare_dram_parameter("d_va", [128, HPC * NT * 128], BF,
                                         isOutput=True)
        d_ot = nc.declare_dram_parameter("d_ot", [128, NG * S], BF,
                                         isOutput=True)
        d_ps = nc.declare_dram_parameter("d_ps", [128, 1024], F32,
                                         isOutput=True)
        d_ps2 = nc.declare_dram_parameter("d_ps2", [128, 1024], F32,
                                          isOutput=True)
        d_po = nc.declare_dram_parameter("d_po", [128, 1024], F32,
                                         isOutput=True)

    with tile.TileContext(nc) as tc:
        with tc.tile_pool(name="singles", bufs=1) as singles:
            wq_sb = singles.tile([128, NK * 256], DT)
            wk_sb = singles.tile([128, NK * 256], DT)
            wv_sb = singles.tile([128, NK * 256], DT)
            wo_sb = singles.tile([128, NG * D], BF)
            # qt/kt: dim-major, bf16. Group g: rows 0:64 = head 2g, rows
            # 64:128 = head 2g+1 (this is what lets QK row-pack the PE).
            qt_sb = singles.tile([128, NG * S], BF)
            kt_sb = singles.tile([128, NG * S], BF)
            ot_sb = singles.tile([128, NG * S], BF)
            # vaug: token-major V, bf16. Head block h, strip j at cols
            # h*2048 + j*128; within a block cols 0:64 = V dims, 64:128 = ones
            # (PV matmul then emits numerator rows 0:64, denominator 64:128).
            vaug_sb = singles.tile([128, HPC * NT * 128], BF)

            # ---- weight DMAs + ones fill --------------------------------
            nc.sync.dma_start(out=wq_sb[:], in_=Wq[:, :])
            nc.sync.dma_start(out=wk_sb[:], in_=Wk[:, :])
            nc.sync.dma_start(out=wv_sb[:], in_=Wv[:, :])
            nc.sync.dma_start(out=wo_sb[:], in_=Wo[:, :])
            for h in range(HPC):
                ones_view = vaug_sb[:, h * 2048:(h + 1) * 2048].rearrange(
                    "p (j b c) -> p j b c", j=NT, b=2, c=64)[:, :, 1, :]
                nc.gpsimd.memset(ones_view, 1.0)

            # ---- phase 1: QKV projections -------------------------------
            # qk_unit / v_unit emit one PSUM tile's worth of projection work
            # (8 accumulating matmuls + one evacuation copy). Chunks 0/1 run
            # up front; chunks 2/3 are emitted as filler units inside the
            # first attention head's strip loop to keep the PE duty cycle
            # high (HAM clock gate) and hide phase-1 time under ACT exp.
            def qk_unit(xc, c, w_sb, dst, g, use_act, pp1):
                ps = pp1.tile([128, 512], F32, name="ps_qk")
                for k in range(NK):
                    nc.tensor.matmul(
                        ps[:],
                        w_sb[:, (k * NG + g) * 128:(k * NG + g + 1) * 128],
                        xc[:, k * 512:(k + 1) * 512],
                        start=(k == 0),
                        stop=(k == NK - 1),
                    )
                dst_ap = dst[:, g * S + c * 512:g * S + (c + 1) * 512]
                if use_act:
                    nc.scalar.copy(out=dst_ap, in_=ps[:])
                else:
                    nc.vector.tensor_copy(out=dst_ap, in_=ps[:])

            def v_unit(xc, c, t, ppv):
                j = c * 4 + t
                pv = ppv.tile([128, DL], F32, name="pv")
                for k in range(NK):
                    nc.tensor.matmul(
                        pv[:],
                        xc[:, k * 512 + t * 128:k * 512 + (t + 1) * 128],
                        wv_sb[:, k * DL:(k + 1) * DL],
                        start=(k == 0),
                        stop=(k == NK - 1),
                    )
                dst = vaug_sb[:].rearrange(
                    "p (h j b c) -> p h j b c",
                    h=HPC, j=NT, b=2, c=64)[:, :, j, 0, :]
                nc.vector.tensor_copy(
                    out=dst, in_=pv[:].rearrange("p (h c) -> p h c", h=HPC))

            def load_chunk(c, xcp):
                xc = xcp.tile([128, 4096], DT, name="xc")
                nc.sync.dma_start(out=xc[:], in_=xC[:, c * 4096:(c + 1) * 4096])
                return xc

            # ---- phase 2/3 emitters -------------------------------------
            def emit_pv(pO, expst, h, j):
                vb = h * 2048 + j * 128
                for sc in range(2):
                    nc.tensor.matmul(
                        pO[:, sc * 512:(sc + 1) * 512],
                        vaug_sb[:, vb:vb + 128],
                        expst[:, sc * 512:(sc + 1) * 512],
                        start=(j == 0),
                        stop=(j == NT - 1),
                        skip_group_check=True,
                    )

            def emit_norm(normp, pO, h, sh):
                g, r = h // 2, (h % 2) * DH
                rb = normp.tile([DH, 1024], F32, name="rb")
                # stock DVE reciprocal is bit-exact iterative divide
                # (~6 cyc/elem); reciprocal_approx_fast mis-executes in
                # this environment (custom ucode row unavailable).
                nc.vector.reciprocal(rb[:], pO[DH:128, :])
                nc.vector.tensor_tensor(
                    out=ot_sb[r:r + DH,
                              g * S + sh * 1024:g * S + (sh + 1) * 1024],
                    in0=pO[0:DH, :],
                    in1=rb[:],
                    op=MULT,
                )

            def ph3_unit(t, pYp, ysbp):
                pY = pYp.tile([128, 1024], F32, name="pS")
                for e in range(2):
                    for g in range(NG):
                        nc.tensor.matmul(
                            pY[:, e * 512:(e + 1) * 512],
                            ot_sb[:, g * S + t * 128:g * S + (t + 1) * 128],
                            wo_sb[:, g * D + e * 512:g * D + (e + 1) * 512],
                            start=(g == 0),
                            stop=(g == NG - 1),
                        )
                ysb = ysbp.tile([128, 1024], F32, name="ysb")
                nc.vector.tensor_copy(out=ysb[:], in_=pY[:])
                nc.sync.dma_start(out=Yp[t * 128:(t + 1) * 128, :], in_=ysb[:])

            def ph2_head(h, sh, pSp, pOp, expp, normp, filler=()):
                filler = list(filler)
                pr, hb = h // 2, (h % 2) * 64
                pO = pOp.tile([128, 1024], F32, name="pO")
                prev = None
                for j in range(NT):
                    pS = pSp.tile([128, 1024], F32, name="pS")
                    for sc in range(2):
                        q0 = pr * S + sh * 1024 + sc * 512
                        nc.tensor.matmul(
                            pS[:, sc * 512:(sc + 1) * 512],
                            kt_sb[hb:hb + 64,
                                  pr * S + j * 128:pr * S + (j + 1) * 128],
                            qt_sb[hb:hb + 64, q0:q0 + 512],
                        )
                    expst = expp.tile([128, 1024], BF, name="expst")
                    nc.scalar.activation(expst[:], pS[:], EXP)
                    if prev is not None:
                        emit_pv(pO, prev, h, j - 1)
                    prev = expst
                    if filler:
                        filler.pop(0)()
                emit_pv(pO, prev, h, NT - 1)
                for f in filler:
                    f()
                emit_norm(normp, pO, h, sh)

            with tc.tile_pool(name="expp", bufs=4) as expp, \
                 tc.tile_pool(name="normp", bufs=2) as normp:
                with tc.tile_pool(name="xcp", bufs=2) as xcp, \
                     tc.tile_pool(name="pp1", bufs=3, space="PSUM") as pp1, \
                     tc.tile_pool(name="ppv", bufs=2, space="PSUM") as ppv:
                    for c in range(NC):
                        xc = load_chunk(c, xcp)
                        for w_sb, dst in ((wq_sb, qt_sb), (wk_sb, kt_sb)):
                            for g in range(NG):
                                qk_unit(xc, c, w_sb, dst, g, True, pp1)
                        for t in range(4):
                            v_unit(xc, c, t, ppv)
                with tc.tile_pool(name="pS", bufs=2, space="PSUM") as pSp, \
                     tc.tile_pool(name="pO", bufs=2, space="PSUM") as pOp, \
                     tc.tile_pool(name="ysbp", bufs=2) as ysbp:
                    for h in range(HPC):
                        ph2_head(h, 0, pSp, pOp, expp, normp)
                    ph3f = [lambda t=t: ph3_unit(t, pSp, ysbp)
                            for t in range(8)]
                    ph2_head(0, 1, pSp, pOp, expp, normp, ph3f)
                    for h in (1, 2, 3):
                        ph2_head(h, 1, pSp, pOp, expp, normp)
                    for t in range(8, 16):
                        ph3_unit(t, pSp, ysbp)
                if dumps:
                    nc.sync.dma_start(out=d_qt[:, :], in_=qt_sb[:])
                    nc.sync.dma_start(out=d_kt[:, :], in_=kt_sb[:])
                    nc.sync.dma_start(out=d_va[:, :], in_=vaug_sb[:])
                    nc.sync.dma_start(out=d_ot[:, :], in_=ot_sb[:])
    nc.finalize()
    return nc


def make_in_maps(x, Wq, Wk, Wv, Wo):
    f = np.float32
    import ml_dtypes
    bf = ml_dtypes.bfloat16
    x = np.asarray(x, f)
    Wq, Wk, Wv, Wo = (np.asarray(a, f) for a in (Wq, Wk, Wv, Wo))
    in_maps = []
    # xC: chunk-major k-strips of x^T: xC[p, c*4096 + k*512 + j]
    #   = x[b, c*512 + j, k*128 + p]
    xCs = []
    for b in range(B):
        xT = np.ascontiguousarray(x[b].T)               # [1024, 2048]
        xC = xT.reshape(NK, 128, NC, 512).transpose(1, 2, 0, 3)
        xCs.append(np.ascontiguousarray(xC.reshape(128, NC * 4096)))

    def wsw(W):  # [1024, 256] -> [128, 2048], block (k*2+g)*128
        return np.ascontiguousarray(
            W.reshape(NK, 128, NG, 128).transpose(1, 0, 2, 3)
            .reshape(128, NK * 256))

    for c in range(N_CORES):
        b, hb = divmod(c, N_CORES // B)
        cols = slice(hb * DL, (hb + 1) * DL)
        wo = (Wo[cols, :] * f(1.0 / 32.0)).reshape(NG, 128, D)
        wo = np.ascontiguousarray(
            wo.transpose(1, 0, 2).reshape(128, NG * D)).astype(bf)
        in_maps.append({
            "xC": xCs[b],
            "Wq": wsw(Wq[:, cols]),
            "Wk": wsw(Wk[:, cols]),
            "Wv": wsw(Wv[:, cols]),
            "Wo": wo,
        })
    return in_maps


def run(inputs, trace=False):
    nc = build_nc()
    in_maps = make_in_maps(**inputs)
    res = run_bass_kernel_spmd(nc, in_maps, list(range(N_CORES)), trace=trace)
    yps = [res.results[c]["Yp"] for c in range(N_CORES)]
    out = np.empty((B, S, D), np.float32)
    cpb = N_CORES // B
    for b in range(B):
        out[b] = sum(yps[b * cpb:(b + 1) * cpb])
    return out, res


def kernel(**inputs):
    out, _ = run(inputs, trace=False)
    return out
